# revision 1
# baseline (speedup 1.0000x reference)
"""ClassCapsule dynamic-routing kernel for 8 Trainium2 NeuronCores.

Problem (hardcoded shapes):
    x:    [64, 2048, 16]  fp32
    W:    [2048, 16, 1024] fp32
    bias: [64, 16]        fp32
    out:  [64, 64, 16]    fp32  (squeezed v after 3 routing iterations)

Strategy (batch-sharded, no collectives):
  - B=64 split across 8 cores (8 batches each).
  - u_hat = einsum('bij,ijk->bik') computed on the PE via a block-diagonal
    trick: 8 in_caps share one matmul; lhsT is a host-prepared block-diagonal
    arrangement of x with K=(i_sub,e)=128, M=(i_sub,b)=64.
  - u_hat tiles [128=(i_sub16,b8), 1024] stored to DRAM (bf16), re-read for
    the 2 remaining routing iterations.
  - Routing per tile: agreement = reduce_d(u_hat*v) (vector), softmax over
    n_caps (ACT exp + vector reciprocal), weighted sum over in_caps via a
    constant 0/1 selector matmul on the PE accumulating in PSUM.
"""

import numpy as np

import concourse.bass as bass
import concourse.tile as tile
from concourse import bacc, mybir
from concourse.bass_utils import run_bass_kernel_spmd

# ---------------------------------------------------------------- constants
B, IC, E = 64, 2048, 16          # batch, in_caps, in_dim
NCAP, D = 64, 16                 # n_caps, cap_dim
ND = NCAP * D                    # 1024
CORES = 8
BL = B // CORES                  # 8 local batches
IB8 = IC // 8                    # 256 blocks of 8 in_caps (matmul granularity)
NT = IC // 16                    # 128 u_hat tiles of 16 in_caps
EPS = 1e-7

FP = mybir.dt.float32
BF = mybir.dt.bfloat16


def _host_prep(x, W, bias):
    """Build per-core host-side tensors."""
    # Block-diagonal x for the projection matmuls.
    # lhsT[blk][(i_sub*16+e), (j_sub*8+b)] = x[b, blk*8+j_sub, e] * (i_sub==j_sub)
    # -> per core: [IB8, 128, 64] fp32
    w_r = W.reshape(IB8, 8 * E, ND)  # [256, 128, 1024]
    # wx[blk] = [128, 1024 + 64]: W block columns then block-diagonal x columns,
    # so ONE dma per block feeds both matmul operands (single sync wait on PE).
    wx_all = []
    for c in range(CORES):
        xc = x[c * BL:(c + 1) * BL]                      # [8, 2048, 16]
        wx = np.zeros((IB8, 128, ND + 8 * BL), dtype=np.float32)
        wx[:, :, :ND] = w_r
        # fill diagonal blocks: rows i_sub*16+e, cols ND + i_sub*8+b
        xr = xc.transpose(1, 2, 0).reshape(IB8, 8, E, BL)  # [blk, i_sub, e, b]
        for s in range(8):
            wx[:, s * E:(s + 1) * E, ND + s * BL:ND + (s + 1) * BL] = xr[:, s]
        wx_all.append(wx)

    # selector: sel8[p, b] = 1 if p % 8 == b   (partition p = i_sub*8 + b)
    sel8 = np.zeros((128, BL), dtype=np.float32)
    sel8[np.arange(128), np.arange(128) % BL] = 1.0

    bias_f = np.tile(bias.reshape(1, ND), (BL, 1)).astype(np.float32)  # [8, 1024]
    return wx_all, sel8, bias_f


def _build_program():
    nc = bacc.Bacc("TRN2", target_bir_lowering=False)

    wx_d = nc.dram_tensor("wx", [IB8, 128, ND + 8 * BL], FP, kind="ExternalInput")
    sel8_d = nc.dram_tensor("sel8", [128, BL], FP, kind="ExternalInput")
    bias_d = nc.dram_tensor("bias_f", [BL, ND], FP, kind="ExternalInput")
    v_out = nc.dram_tensor("v_out", [BL, ND], FP, kind="ExternalOutput")

    u_hat_d = nc.dram_tensor("u_hat_d", [NT, 128, ND], BF)   # internal scratch
    v_scr = nc.dram_tensor("v_scr", [BL, ND], BF)            # bcast bounce

    with tile.TileContext(nc) as tc:
        with (
            tc.tile_pool(name="wp", bufs=4) as wp,
            tc.tile_pool(name="up", bufs=3) as up,
            tc.tile_pool(name="tp", bufs=3) as tp,
            tc.tile_pool(name="smalls", bufs=4) as sp,
            tc.tile_pool(name="consts", bufs=1) as cp,
            tc.tile_pool(name="vb", bufs=2) as vbp,
            tc.tile_pool(name="ps", bufs=2, space="PSUM") as psp,
            tc.tile_pool(name="ps_acc", bufs=1, space="PSUM") as psa,
            tc.tile_pool(name="bstate", bufs=1) as bsp,
        ):
            # ---- constants resident in SBUF
            sel8_f = cp.tile([128, BL], FP)
            nc.sync.dma_start(out=sel8_f, in_=sel8_d[:, :])
            sel8_b = cp.tile([128, BL], BF)
            nc.scalar.copy(out=sel8_b, in_=sel8_f)
            bias_sb = cp.tile([BL, ND], FP)
            nc.sync.dma_start(out=bias_sb, in_=bias_d[:, :])
            eps_t = cp.tile([BL, 1], FP)
            nc.vector.memset(eps_t, EPS)

            # routing logits state: [128, NT*64]
            b_all = bsp.tile([128, NT * NCAP], FP)

            # ---------------- squash helper: v = squash(s_psum*scale + bias)
            def squash_from_psum(s_ps, scale):
                s_sb = sp.tile([BL, ND], FP, tag="s_sb")
                # s = s_ps*scale + bias
                nc.vector.scalar_tensor_tensor(
                    out=s_sb, in0=s_ps, scalar=float(scale), in1=bias_sb,
                    op0=mybir.AluOpType.mult, op1=mybir.AluOpType.add)
                sq = sp.tile([BL, ND], FP, tag="sq")
                nc.vector.tensor_mul(sq, s_sb, s_sb)
                nsq = sp.tile([BL, NCAP], FP, tag="nsq")
                nc.vector.reduce_sum(
                    out=nsq, in_=sq.rearrange("p (n d) -> p n d", d=D),
                    axis=mybir.AxisListType.X)
                norm = sp.tile([BL, NCAP], FP, tag="norm")
                # norm = sqrt(nsq + EPS)
                nc.scalar.activation(out=norm, in_=nsq,
                                     func=mybir.ActivationFunctionType.Sqrt,
                                     bias=eps_t[:, :], scale=1.0)
                den = sp.tile([BL, NCAP], FP, tag="den")
                # den = (nsq + EPS + 1) * norm
                nc.vector.scalar_tensor_tensor(
                    out=den, in0=nsq, scalar=float(EPS + 1.0), in1=norm,
                    op0=mybir.AluOpType.add, op1=mybir.AluOpType.mult)
                rden = sp.tile([BL, NCAP], FP, tag="rden")
                nc.vector.reciprocal(out=rden, in_=den)
                fac = sp.tile([BL, NCAP], FP, tag="fac")
                # fac = (nsq + EPS) * rden
                nc.vector.scalar_tensor_tensor(
                    out=fac, in0=nsq, scalar=float(EPS), in1=rden,
                    op0=mybir.AluOpType.add, op1=mybir.AluOpType.mult)
                v_sb = sp.tile([BL, ND], FP, tag="v_sb")
                fac_b = bass.AP(tensor=fac.tensor, offset=fac.offset,
                                ap=[list(fac.ap[0]), list(fac.ap[1]), [0, D]])
                nc.vector.tensor_mul(
                    v_sb.rearrange("p (n d) -> p n d", d=D),
                    s_sb.rearrange("p (n d) -> p n d", d=D),
                    fac_b)
                return s_sb, v_sb

            def broadcast_v(v_sb):
                """v_sb [8, 1024] fp32 -> vb [128, 1024] bf16 (partition bcast)."""
                v_bf = sp.tile([BL, ND], BF, tag="v_bf")
                nc.vector.tensor_copy(out=v_bf, in_=v_sb)
                nc.sync.dma_start(out=v_scr[:, :], in_=v_bf)
                vb = vbp.tile([128, ND], BF, tag="vb")
                src = bass.AP(tensor=v_scr, offset=0,
                              ap=[[0, 128 // BL], [ND, BL], [1, ND]])
                nc.sync.dma_start(out=vb, in_=src)
                return vb

            # ================= Phase P: projection + iter-0 sum =================
            s0_ps = psa.tile([BL, ND], FP, tag="s_acc")
            for t in range(NT):
                u_ps = psp.tile([128, ND], FP, tag="u_ps")
                for h in range(2):  # two 8-in_cap blocks -> partitions h*64..
                    blk = 2 * t + h
                    wt = wp.tile([128, ND + 8 * BL], FP, tag="w")
                    nc.sync.dma_start(out=wt, in_=wx_d[blk])
                    for nh in range(2):  # N halves of 512
                        nc.tensor.matmul(
                            u_ps[h * 64:(h + 1) * 64, nh * 512:(nh + 1) * 512],
                            wt[:, ND:ND + 8 * BL],
                            wt[:, nh * 512:(nh + 1) * 512],
                            start=True, stop=True)
                u_bf = up.tile([128, ND], BF, tag="u_bf")
                nc.scalar.copy(out=u_bf[:, 0:512], in_=u_ps[:, 0:512])
                nc.scalar.copy(out=u_bf[:, 512:1024], in_=u_ps[:, 512:1024])
                nc.sync.dma_start(out=u_hat_d[t], in_=u_bf)
                for nh in range(2):
                    nc.tensor.matmul(
                        s0_ps[:, nh * 512:(nh + 1) * 512],
                        sel8_b, u_bf[:, nh * 512:(nh + 1) * 512],
                        start=(t == 0), stop=(t == NT - 1),
                        skip_group_check=True)

            _, v_sb = squash_from_psum(s0_ps, 1.0 / NCAP)
            vb = broadcast_v(v_sb)

            # ================= Routing iterations 1 and 2 =================
            for it in (1, 2):
                s_ps = psa.tile([BL, ND], FP, tag="s_acc")
                for t in range(NT):
                    u_bf = up.tile([128, ND], BF, tag="u_bf")
                    nc.sync.dma_start(out=u_bf, in_=u_hat_d[t])
                    tmp = tp.tile([128, ND], BF, tag="tmp")
                    nc.gpsimd.tensor_mul(tmp, u_bf, vb)
                    b_slice = b_all[:, t * NCAP:(t + 1) * NCAP]
                    if it == 1:
                        # b starts at zero: agreement goes straight into b
                        nc.vector.reduce_sum(
                            out=b_slice,
                            in_=tmp.rearrange("p (n d) -> p n d", d=D),
                            axis=mybir.AxisListType.X)
                    else:
                        agr = sp.tile([128, NCAP], FP, tag="agr")
                        nc.vector.reduce_sum(
                            out=agr,
                            in_=tmp.rearrange("p (n d) -> p n d", d=D),
                            axis=mybir.AxisListType.X)
                        nc.vector.tensor_add(b_slice, b_slice, agr)
                    c_un = sp.tile([128, NCAP], FP, tag="c_un")
                    se = sp.tile([128, 1], FP, tag="se")
                    nc.scalar.activation(out=c_un, in_=b_slice,
                                         func=mybir.ActivationFunctionType.Exp,
                                         accum_out=se)
                    rec = sp.tile([128, 1], FP, tag="rec")
                    nc.vector.reciprocal(out=rec, in_=se)
                    c_bf = sp.tile([128, NCAP], BF, tag="c_bf")
                    nc.scalar.mul(c_bf, c_un, rec)
                    w_bf = tp.tile([128, ND], BF, tag="w_bf")
                    c_b = bass.AP(tensor=c_bf.tensor, offset=c_bf.offset,
                                  ap=[list(c_bf.ap[0]), list(c_bf.ap[1]), [0, D]])
                    nc.vector.tensor_mul(
                        w_bf.rearrange("p (n d) -> p n d", d=D),
                        u_bf.rearrange("p (n d) -> p n d", d=D),
                        c_b)
                    for nh in range(2):
                        nc.tensor.matmul(
                            s_ps[:, nh * 512:(nh + 1) * 512],
                            sel8_b, w_bf[:, nh * 512:(nh + 1) * 512],
                            start=(t == 0), stop=(t == NT - 1),
                            skip_group_check=True)
                _, v_sb = squash_from_psum(s_ps, 1.0)
                if it < 2:
                    vb = broadcast_v(v_sb)
                else:
                    nc.sync.dma_start(out=v_out[:, :], in_=v_sb)

    nc.compile()
    return nc


_CACHED = {}


def _get_program():
    if "nc" not in _CACHED:
        _CACHED["nc"] = _build_program()
    return _CACHED["nc"]


def kernel(x, W, bias):
    x = np.asarray(x, dtype=np.float32)
    W = np.asarray(W, dtype=np.float32)
    bias = np.asarray(bias, dtype=np.float32)

    wx_all, sel8, bias_f = _host_prep(x, W, bias)
    nc = _get_program()

    in_maps = []
    for c in range(CORES):
        in_maps.append({
            "wx": wx_all[c],
            "sel8": sel8,
            "bias_f": bias_f,
        })
    res = run_bass_kernel_spmd(nc, in_maps, core_ids=list(range(CORES)))
    _CACHED["last_results"] = res
    outs = [res.results[c]["v_out"].reshape(BL, NCAP, D) for c in range(CORES)]
    return np.concatenate(outs, axis=0)



# revision 12
# speedup vs baseline: 1.9798x; 1.9798x over previous
"""ClassCapsule dynamic-routing kernel for 8 Trainium2 NeuronCores.

Problem (hardcoded shapes):
    x:    [64, 2048, 16]  fp32
    W:    [2048, 16, 1024] fp32
    bias: [64, 16]        fp32
    out:  [64, 64, 16]    fp32  (squeezed v after 3 routing iterations)

Strategy (in_caps-sharded, W resident in SBUF, u_hat recomputed per
iteration, per-iteration AllReduce of the small s tensor):
  - in_caps=2048 split across 8 cores (256 each); every core holds the
    full batch B=64.  W slice (bf16) lives in SBUF for the whole kernel,
    so u_hat is recomputed on the PE each routing iteration instead of
    being bounced through DRAM.  Total HBM traffic is ~15 MB/core.
  - u_hat tiles [128=(i8,b16), 1024=(d16,n64)] come from block-diagonal
    matmuls: lhsT = block-diag x (8 in_caps share K=128=(i8,e16)),
    rhs = W block.  Column order (d major, n minor) keeps the free-dim
    broadcast of c packed so DVE runs in 2x bf16 mode.
  - iteration 0 (uniform c): s0 = sum_i u/64 collapses into a dense
    x^T @ W matmul - no u_hat materialization at all.
  - routing: agreement = u*v reduced over d via a halving add tree
    (DVE, bf16), softmax over n (ACT exp + DVE), weighted sum over i
    via selector matmuls on the PE accumulating in PSUM.
  - s [64,1024] partials are AllReduced (collective_compute) across the
    8 cores each iteration; every core computes squash/v redundantly.
"""

import numpy as np
import ml_dtypes

import concourse.bass as bass
import concourse.tile as tile
from concourse import bacc, mybir
from concourse.bass_utils import run_bass_kernel_spmd

# ---------------------------------------------------------------- constants
B, IC, E = 64, 2048, 16          # batch, in_caps, in_dim
NCAP, D = 64, 16                 # n_caps, cap_dim
ND = NCAP * D                    # 1024
CORES = 8
ICL = IC // CORES                # 256 local in_caps
NB = ICL // 8                    # 32 blocks of 8 in_caps
BC = 4                           # batch chunks of 16
EPS = 1e-7

FP = mybir.dt.float32
BF = mybir.dt.bfloat16
BF_NP = ml_dtypes.bfloat16


def _host_prep(x, W, bias):
    """Per-core host-side tensors (bf16, (d,n) column order)."""
    # W columns reordered from (n,d) to (d,n): new_col = d*64 + n
    W_dn = W.reshape(IC, E, NCAP, D).transpose(0, 1, 3, 2).reshape(IC, E, ND)

    w_all, xbd_all, xd_all = [], [], []
    for c in range(CORES):
        sl = slice(c * ICL, (c + 1) * ICL)
        W_c = W_dn[sl]                                   # [256, 16, 1024]
        # -> [128=(i8,e16) partitions, 32 blocks, 1024]
        w_all.append(np.ascontiguousarray(
            W_c.reshape(NB, 8, E, ND).transpose(1, 2, 0, 3).reshape(128, NB, ND)
        ).astype(BF_NP))

        x_c = x[:, sl]                                   # [64, 256, 16]
        # block-diagonal lhsT: [128=(i8,e16), blk, bc, 128=(i8,b16)]
        x_r = x_c.reshape(BC, 16, NB, 8, E).transpose(3, 4, 2, 0, 1)
        arr = np.zeros((8, E, NB, BC, 8, 16), dtype=np.float32)
        for s in range(8):
            arr[s, :, :, :, s, :] = x_r[s]
        xbd_all.append(arr.reshape(128, NB, BC, 128).astype(BF_NP))

        # dense lhsT for iter-0 direct sum: [128=(i8,e16), blk, 128(m: b64 pad)]
        xd = np.zeros((128, NB, 128), dtype=np.float32)
        xd[:, :, :B] = x_c.reshape(B, NB, 8, E).transpose(2, 3, 1, 0).reshape(128, NB, B)
        xd_all.append(xd.astype(BF_NP))

    # selector weights, one per batch chunk: sel[bc][k=(i8,b16), m=128] with
    # m = bc*16 + (k%16) set to 1  (M=128 keeps free-weight-load enabled)
    sels = np.zeros((BC, 128, 128), dtype=np.float32)
    for bc in range(BC):
        k = np.arange(128)
        sels[bc, k, bc * 16 + (k % 16)] = 1.0
    sels = np.ascontiguousarray(sels.transpose(1, 0, 2)).astype(BF_NP)  # [128, BC, 128]

    # bias in (d,n) order, tiled over batch: [64, 1024]
    bias_dn = np.ascontiguousarray(bias.T).reshape(1, ND)       # [d,n] flat
    bias_f = np.tile(bias_dn, (B, 1)).astype(np.float32)
    return w_all, xbd_all, xd_all, sels, bias_f


def _build_program():
    nc = bacc.Bacc("TRN2", target_bir_lowering=False, num_devices=CORES)

    w_d = nc.dram_tensor("w_d", [128, NB, ND], BF, kind="ExternalInput")
    xbd_d = nc.dram_tensor("xbd_d", [128, NB, BC, 128], BF, kind="ExternalInput")
    xd_d = nc.dram_tensor("xd_d", [128, NB, 128], BF, kind="ExternalInput")
    sel_d = nc.dram_tensor("sel_d", [128, BC, 128], BF, kind="ExternalInput")
    bias_d = nc.dram_tensor("bias_d", [B, ND], FP, kind="ExternalInput")
    v_out = nc.dram_tensor("v_out", [B, ND], FP, kind="ExternalOutput")

    v_scr = nc.dram_tensor("v_scr", [B, ND], BF)     # bounce for vb build

    with tile.TileContext(nc) as tc:
        with (
            tc.tile_pool(name="consts", bufs=1) as cp,
            tc.tile_pool(name="ubf", bufs=2) as up,       # [128, 4096] grouped
            tc.tile_pool(name="tmp", bufs=1) as tp,       # [128, 4096] grouped
            tc.tile_pool(name="wsb", bufs=2) as wp,       # [128, 4096] grouped
            tc.tile_pool(name="vb", bufs=1) as vbp,
            tc.tile_pool(name="smalls", bufs=2) as sp,
            tc.tile_pool(name="sq", bufs=1) as qp,
            tc.tile_pool(name="ups", bufs=3, space="PSUM") as psp,
            tc.tile_pool(name="sps", bufs=1, space="PSUM") as psa,
            tc.tile_pool(name="bstate", bufs=1) as bsp,
            tc.tile_pool(name="dram", bufs=2, space="DRAM") as dp,
        ):
            # ---- resident tensors
            w_sb = cp.tile([128, NB * ND], BF)
            nc.sync.dma_start(out=w_sb, in_=w_d[:, :, :])
            xbd_sb = cp.tile([128, NB * BC * 128], BF)
            nc.sync.dma_start(out=xbd_sb, in_=xbd_d[:, :, :, :])
            xd_sb = cp.tile([128, NB * 128], BF)
            nc.sync.dma_start(out=xd_sb, in_=xd_d[:, :, :])
            sel_sb = cp.tile([128, BC * 128], BF)
            nc.sync.dma_start(out=sel_sb, in_=sel_d[:, :, :])
            bias_sb = cp.tile([B, ND], FP)
            nc.sync.dma_start(out=bias_sb, in_=bias_d[:, :])
            eps_t = cp.tile([B, 1], FP)
            nc.vector.memset(eps_t, EPS)

            # routing logits state: [128=(i8,b16), bc*blk*n]
            b_all = bsp.tile([128, BC * NB * NCAP], FP)

            def w_blk(blk, h):
                return w_sb[:, blk * ND + h * 512:blk * ND + (h + 1) * 512]

            # ---------------- AllReduce s -> (scale,bias) -> squash -> v
            def reduce_squash_v(s_ps, scale, last):
                s_par = qp.tile([B, ND], FP, tag="q1")
                nc.vector.tensor_copy(out=s_par, in_=s_ps[0:B, :])
                s_in = dp.tile([B, ND], FP, tag="cc_in")
                nc.sync.dma_start(out=s_in[:], in_=s_par)
                s_red = dp.tile([B, ND], FP, tag="cc_out")
                nc.gpsimd.collective_compute(
                    "AllReduce",
                    mybir.AluOpType.add,
                    replica_groups=[list(range(CORES))],
                    ins=[s_in[:].opt()],
                    outs=[s_red[:].opt()],
                )
                s_glob = qp.tile([B, ND], FP, tag="q2")
                nc.sync.dma_start(out=s_glob, in_=s_red[:])
                # s = s_glob*scale + bias
                s_sb = qp.tile([B, ND], FP, tag="q1")
                nc.vector.scalar_tensor_tensor(
                    out=s_sb, in0=s_glob, scalar=float(scale), in1=bias_sb,
                    op0=mybir.AluOpType.mult, op1=mybir.AluOpType.add)
                sqr = qp.tile([B, ND], FP, tag="q2")
                nc.scalar.square(out=sqr, in_=s_sb)
                nsq = sp.tile([B, NCAP], FP, tag="nsq")
                nc.vector.reduce_sum(
                    out=nsq, in_=sqr.rearrange("p (d n) -> p n d", d=D),
                    axis=mybir.AxisListType.X)
                norm = sp.tile([B, NCAP], FP, tag="norm")
                nc.scalar.activation(out=norm, in_=nsq,
                                     func=mybir.ActivationFunctionType.Sqrt,
                                     bias=eps_t[:, :], scale=1.0)
                den = sp.tile([B, NCAP], FP, tag="den")
                nc.vector.scalar_tensor_tensor(
                    out=den, in0=nsq, scalar=float(EPS + 1.0), in1=norm,
                    op0=mybir.AluOpType.add, op1=mybir.AluOpType.mult)
                rden = sp.tile([B, NCAP], FP, tag="rden")
                nc.vector.reciprocal(out=rden, in_=den)
                fac = sp.tile([B, NCAP], FP, tag="fac")
                nc.vector.scalar_tensor_tensor(
                    out=fac, in0=nsq, scalar=float(EPS), in1=rden,
                    op0=mybir.AluOpType.add, op1=mybir.AluOpType.mult)
                v_sb = qp.tile([B, ND], FP, tag="q2")
                fac_b = bass.AP(tensor=fac.tensor, offset=fac.offset,
                                ap=[list(fac.ap[0]), [0, D], list(fac.ap[1])])
                nc.vector.tensor_mul(
                    v_sb.rearrange("p (d n) -> p d n", d=D),
                    s_sb.rearrange("p (d n) -> p d n", d=D),
                    fac_b)
                if last:
                    nc.sync.dma_start(out=v_out[:, :], in_=v_sb)
                    return None
                v_bf = qp.tile([B, ND], BF, tag="q3")
                nc.vector.tensor_copy(out=v_bf, in_=v_sb)
                nc.sync.dma_start(out=v_scr[:, :], in_=v_bf)
                # vb_all[128=(i8,b16), (bc,d,n)]: v rows bc*16+b_lo, bcast i8
                vb = vbp.tile([128, BC * ND], BF, tag="vb")
                for bc in range(BC):
                    src = bass.AP(tensor=v_scr, offset=bc * 16 * ND,
                                  ap=[[0, 8], [ND, 16], [1, ND]])
                    nc.sync.dma_start(
                        out=vb[:, bc * ND:(bc + 1) * ND], in_=src)
                return vb

            # ================= iter 0: s0 = (1/64) sum_i u  ================
            s_ps = psa.tile([128, ND], FP, tag="s_acc")
            for blk in range(NB):
                for h in range(2):
                    nc.tensor.matmul(
                        s_ps[:, h * 512:(h + 1) * 512],
                        xd_sb[:, blk * 128:(blk + 1) * 128],
                        w_blk(blk, h),
                        start=(blk == 0), stop=(blk == NB - 1))
            vb = reduce_squash_v(s_ps, 1.0 / NCAP, last=False)

            # ================= routing iterations 1 and 2 =================
            for it in (1, 2):
                s_ps = psa.tile([128, ND], FP, tag="s_acc")
                for blk in range(NB):
                    u_g = up.tile([128, BC * ND], BF, tag="u_g")
                    for bc in range(BC):
                        u_ps = psp.tile([128, ND], FP, tag="u_ps")
                        lhs = xbd_sb[:, (blk * BC + bc) * 128:
                                     (blk * BC + bc + 1) * 128]
                        for h in range(2):
                            nc.tensor.matmul(
                                u_ps[:, h * 512:(h + 1) * 512],
                                lhs, w_blk(blk, h),
                                start=True, stop=True)
                        nc.scalar.copy(
                            out=u_g[:, bc * ND:(bc + 1) * ND], in_=u_ps)
                    # tmp = u * v  (bf16, packed -> 2x DVE)
                    tmp = tp.tile([128, BC * ND], BF, tag="tmp")
                    nc.vector.tensor_mul(tmp, u_g, vb)
                    # reduce over d: halving add tree on [p, bc, (d n)]
                    t3 = tmp.rearrange("p (c f) -> p c f", c=BC)
                    for half in (512, 256, 128, 64):
                        src_hi = bass.AP(
                            tensor=tmp.tensor, offset=tmp.offset + half,
                            ap=[list(tmp.ap[0]), [ND, BC], [1, half]])
                        if half > 64:
                            nc.vector.tensor_add(
                                t3[:, :, 0:half], t3[:, :, 0:half], src_hi)
                        else:
                            # final add -> b state (fp32)
                            b_dst = bass.AP(
                                tensor=b_all.tensor,
                                offset=b_all.offset + blk * NCAP,
                                ap=[list(b_all.ap[0]), [NB * NCAP, BC],
                                    [1, NCAP]])
                            if it == 1:
                                nc.vector.tensor_add(
                                    b_dst, t3[:, :, 0:64], src_hi)
                            else:
                                agr = sp.tile([128, BC * NCAP], FP, tag="agr")
                                a3 = agr.rearrange("p (c n) -> p c n", c=BC)
                                nc.vector.tensor_add(
                                    a3, t3[:, :, 0:64], src_hi)
                                nc.vector.tensor_add(b_dst, b_dst, a3)
                    # softmax over n (free axis): exp, Z, recip, scale
                    b_src = bass.AP(
                        tensor=b_all.tensor,
                        offset=b_all.offset + blk * NCAP,
                        ap=[list(b_all.ap[0]), [NB * NCAP, BC], [1, NCAP]])
                    c_un = sp.tile([128, BC * NCAP], BF, tag="c_un")
                    nc.scalar.activation(
                        out=c_un.rearrange("p (c n) -> p c n", c=BC),
                        in_=b_src, func=mybir.ActivationFunctionType.Exp)
                    zsum = sp.tile([128, BC], FP, tag="zsum")
                    nc.vector.reduce_sum(
                        out=zsum, in_=c_un.rearrange("p (c n) -> p c n", c=BC),
                        axis=mybir.AxisListType.X)
                    rec = sp.tile([128, BC], BF, tag="rec")
                    with nc.allow_low_precision(reason="1/Z in bf16 is fine for softmax scale"):
                        nc.vector.reciprocal(out=rec, in_=zsum)
                    c_bf = sp.tile([128, BC * NCAP], BF, tag="c_bf")
                    rec_b = bass.AP(tensor=rec.tensor, offset=rec.offset,
                                    ap=[list(rec.ap[0]), [1, BC], [0, NCAP]])
                    nc.gpsimd.tensor_mul(
                        c_bf.rearrange("p (c n) -> p c n", c=BC),
                        c_un.rearrange("p (c n) -> p c n", c=BC),
                        rec_b)
                    # w = u * c (c bcast over d; last dim packed -> 2x DVE)
                    w_g = wp.tile([128, BC * ND], BF, tag="w_g")
                    c_b = bass.AP(tensor=c_bf.tensor, offset=c_bf.offset,
                                  ap=[list(c_bf.ap[0]), [NCAP, BC], [0, D],
                                      [1, NCAP]])
                    nc.vector.tensor_mul(
                        w_g.rearrange("p (c d n) -> p c d n", c=BC, d=D),
                        u_g.rearrange("p (c d n) -> p c d n", c=BC, d=D),
                        c_b)
                    # s += sel_bc^T w   (accumulate over blocks in PSUM)
                    for bc in range(BC):
                        for h in range(2):
                            nc.tensor.matmul(
                                s_ps[:, h * 512:(h + 1) * 512],
                                sel_sb[:, bc * 128:(bc + 1) * 128],
                                w_g[:, bc * ND + h * 512:bc * ND + (h + 1) * 512],
                                start=(blk == 0 and bc == 0),
                                stop=(blk == NB - 1 and bc == BC - 1),
                                skip_group_check=True)
                vb = reduce_squash_v(s_ps, 1.0, last=(it == 2))

    nc.compile()
    return nc


_CACHED = {}


def _get_program():
    if "nc" not in _CACHED:
        _CACHED["nc"] = _build_program()
    return _CACHED["nc"]


def kernel(x, W, bias):
    x = np.asarray(x, dtype=np.float32)
    W = np.asarray(W, dtype=np.float32)
    bias = np.asarray(bias, dtype=np.float32)

    w_all, xbd_all, xd_all, sels, bias_f = _host_prep(x, W, bias)
    nc = _get_program()

    in_maps = []
    for c in range(CORES):
        in_maps.append({
            "w_d": w_all[c],
            "xbd_d": xbd_all[c],
            "xd_d": xd_all[c],
            "sel_d": sels,
            "bias_d": bias_f,
        })
    res = run_bass_kernel_spmd(nc, in_maps, core_ids=list(range(CORES)))
    _CACHED["last_results"] = res
    # v_out is replicated; columns are (d,n) ordered -> [b, n, d]
    v = res.results[0]["v_out"].reshape(B, D, NCAP).transpose(0, 2, 1)
    return np.ascontiguousarray(v)
